# revision 19
# baseline (speedup 1.0000x reference)
"""AggBondModule kernel v5 for Trainium2 (8 NeuronCores, SPMD edge-sharding).

out[e] = relu(concat(node_feat[src[e]], node_feat[dst[e]], edge_feat[e]) @ W + b)
       = relu(P1b[src[e]] + P2[dst[e]] + edge_feat[e] @ W3)

P1b = node_feat @ W1 + b, P2 = node_feat @ W2: host-projected fp16 node
table P12[n] = [P1b[n] | P2[n]] (512B/row).

Device per 2048-edge block, all edge-major (edge e = blk*2048 + ch*128 + p
lives at [partition p, cols ch*128:(ch+1)*128]):
  - 2x SWDGE dma_gather of 256B fp16 rows (elem_step selects the P1b/P2
    half), round-robin over 4 queues; queues drain concurrently (~140GB/s
    aggregate) once no other small-packet traffic competes.
  - PE: 16x (LoadStationary efT chunk [128f x 128e], moving W3 [128f x 128j])
    -> psum edge-major. No transpose anywhere (v3's DMA_TRANSPOSE emitted
    54k extra 256B packets that throttled the gather queues).
  - Scalar: 4x 512-wide ACT Copy psum -> ep_em fp16.
  - DVE: g1+g2, +ep, relu(max 0); store edge-major fp16.

Host-side 4-class machinery (int16 gather indices address the table in two
halves; edges class-sorted by (src < 32768, dst < 32768)).
"""

import sys

import numpy as np

sys.path.insert(0, "/opt/trn_rl_repo")

P = 128
D = 128
N_NODES = 50000
HALF = 32768
E_TOTAL = 400000
N_CORES = 8
C = 16  # 128-edge chunks per block
BLOCK = P * C  # 2048
CLASS_BLOCKS = (11, 6, 6, 3)
E_LOC = E_TOTAL // N_CORES

F16 = np.float16
N_QUEUES = 4
SCRATCH = 98304


def build_program(
    n_nodes=N_NODES,
    half=HALF,
    c=C,
    class_blocks=CLASS_BLOCKS,
    num_devices=N_CORES,
    n_queues=N_QUEUES,
):
    from concourse import bacc, mybir, tile

    block = P * c
    n_blocks = sum(class_blocks)
    e_pad = block * n_blocks
    idx_words = block // 16
    f32 = mybir.dt.float32
    fp16 = mybir.dt.float16
    i16 = mybir.dt.int16

    nc = bacc.Bacc(
        "TRN2",
        target_bir_lowering=False,
        debug=False,
        num_devices=num_devices,
        num_swdge_queues=n_queues,
        dynamic_dma_scratch_size=SCRATCH,
    )

    p1 = nc.dram_tensor("p1", [n_nodes, D], fp16, kind="ExternalInput").ap()
    p2 = nc.dram_tensor("p2", [n_nodes, D], fp16, kind="ExternalInput").ap()
    ef_t = nc.dram_tensor("efT", [P, e_pad], fp16, kind="ExternalInput").ap()
    w3_dram = nc.dram_tensor("W3", [P, D], fp16, kind="ExternalInput").ap()
    idx_dram = nc.dram_tensor(
        "idx_all", [P, n_blocks * 2 * idx_words], i16, kind="ExternalInput"
    ).ap()
    out_em = nc.dram_tensor("out_em", [n_blocks, P, c, D], fp16, kind="ExternalOutput").ap()

    # table views: [half rows] of the split P1b / P2 tables
    views = {
        (0, 0): p1[:half, :],
        (1, 0): p1[half:, :],
        (0, 1): p2[:half, :],
        (1, 1): p2[half:, :],
    }

    block_cls = []
    for cls_i, nb in enumerate(class_blocks):
        block_cls += [(cls_i >> 1 & 1, cls_i & 1)] * nb

    with tile.TileContext(nc) as tc:
        with (
            tc.tile_pool(name="const", bufs=1) as const_pool,
            tc.tile_pool(name="gath", bufs=6) as gath_pool,
            tc.tile_pool(name="feats", bufs=4) as feat_pool,
            tc.tile_pool(name="work", bufs=3) as work_pool,
            tc.tile_pool(name="opsum", bufs=2, space="PSUM") as ps_pool,
        ):
            # per-block idx loads, first blocks before anything else so the
            # gather pipeline starts immediately
            idx_all = const_pool.tile([P, n_blocks * 2 * idx_words], i16)
            w3_tile = const_pool.tile([P, D], fp16)
            for blk in range(n_blocks):
                lo = (2 * blk) * idx_words
                hi = (2 * blk + 2) * idx_words
                nc.sync.dma_start(idx_all[:, lo:hi], idx_dram[:, lo:hi])
                if blk == 3:
                    nc.sync.dma_start(w3_tile[:], w3_dram[:])

            for blk in range(n_blocks):
                src_hi, dst_hi = block_cls[blk]
                base = blk * block

                sidx = idx_all[:, (2 * blk) * idx_words : (2 * blk + 1) * idx_words]
                didx = idx_all[:, (2 * blk + 1) * idx_words : (2 * blk + 2) * idx_words]

                g1 = gath_pool.tile([P, c, D], fp16, tag="g1")
                g2 = gath_pool.tile([P, c, D], fp16, tag="g2")

                hb = block // 2
                hw_ = idx_words // 2
                for half_i, (g, vw, idxs) in enumerate(
                    ((g1, views[(src_hi, 0)], sidx), (g2, views[(dst_hi, 1)], didx))
                ):
                    for piece in range(2):
                        nc.gpsimd.dma_gather(
                            out_ap=g[:, piece * (c // 2) : (piece + 1) * (c // 2), :],
                            in_ap=vw,
                            idxs_ap=idxs[:, piece * hw_ : (piece + 1) * hw_],
                            num_idxs=hb,
                            num_idxs_reg=hb,
                            elem_size=D,
                            single_packet=False,
                            queue_num=(4 * blk + 2 * half_i + piece) % n_queues,
                        )

                sb_e = feat_pool.tile([P, block], fp16, tag="sb_e")
                nc.sync.dma_start(sb_e[:], ef_t[:, base : base + block])

                # EP = ef @ W3, edge-major: LS per 128-edge chunk, W3 moving
                psum = ps_pool.tile([P, block], f32, tag="ps")
                for ch in range(c):
                    nc.tensor.matmul(
                        psum[:, ch * D : (ch + 1) * D],
                        lhsT=sb_e[:, ch * P : (ch + 1) * P],
                        rhs=w3_tile[:],
                        start=True,
                        stop=True,
                    )
                ep_em = feat_pool.tile([P, block], fp16, tag="ep_em")
                for g in range(block // 512):
                    nc.scalar.activation(
                        ep_em[:, g * 512 : (g + 1) * 512],
                        psum[:, g * 512 : (g + 1) * 512],
                        mybir.ActivationFunctionType.Copy,
                    )

                t1 = work_pool.tile([P, c, D], fp16, tag="t1")
                nc.vector.tensor_tensor(t1[:], g1[:], g2[:], mybir.AluOpType.add)
                out_sb = work_pool.tile([P, c, D], fp16, tag="out_sb")
                nc.vector.tensor_tensor(out_sb[:], t1[:], ep_em[:], mybir.AluOpType.add)
                nc.vector.tensor_scalar_max(out_sb[:], out_sb[:], 0.0)

                nc.sync.dma_start(out_em[blk], out_sb[:])

    nc.compile()
    return nc


_PROGRAM_CACHE = {}


def _get_program(class_blocks=CLASS_BLOCKS):
    key = tuple(class_blocks)
    if key not in _PROGRAM_CACHE:
        _PROGRAM_CACHE[key] = build_program(class_blocks=key)
    return _PROGRAM_CACHE[key]


def make_tables(node_feat, W, b):
    """P1b[n] = node_feat[n] @ W1 + b, P2[n] = node_feat[n] @ W2, fp16."""
    p1 = node_feat @ W[:D, :] + b
    p2 = node_feat @ W[D : 2 * D, :]
    return (
        np.ascontiguousarray(p1.astype(F16)),
        np.ascontiguousarray(p2.astype(F16)),
    )


def shard_core(edge_feat_c, src_c, dst_c, class_blocks, half=HALF, c=C):
    """Classify/permute/pad one core's edges. Returns fp16 efT + wrapped idx."""
    block = P * c
    idx_words = block // 16
    n_blocks = sum(class_blocks)
    e_pad = block * n_blocks
    cls = (src_c >= half).astype(np.int64) * 2 + (dst_c >= half).astype(np.int64)
    # sort by (class, src): the src-gather then reads near-ascending table
    # rows (HBM row-buffer locality); dst stays random.
    order = np.lexsort((src_c, cls))
    counts = np.bincount(cls, minlength=4)
    seg_off = np.concatenate([[0], np.cumsum(np.asarray(class_blocks) * block)])

    ef_pad = np.zeros((e_pad, D), F16)
    s_pad = np.zeros((e_pad,), np.int16)
    d_pad = np.zeros((e_pad,), np.int16)
    src_reb = np.where(src_c >= half, src_c - half, src_c).astype(np.int16)
    dst_reb = np.where(dst_c >= half, dst_c - half, dst_c).astype(np.int16)

    pos = 0
    for k in range(4):
        sel = order[pos : pos + counts[k]]
        off = seg_off[k]
        ef_pad[off : off + counts[k]] = edge_feat_c[sel]
        s_pad[off : off + counts[k]] = src_reb[sel]
        d_pad[off : off + counts[k]] = dst_reb[sel]
        pos += counts[k]

    ef_t = np.ascontiguousarray(ef_pad.T)  # [128, e_pad] fp16

    def wrap(v):
        w16 = v.reshape(n_blocks, idx_words, 16).transpose(0, 2, 1)
        return np.ascontiguousarray(np.tile(w16, (1, P // 16, 1)))

    blk_cnt = np.full((n_blocks,), block, np.int32)

    sw, dw = wrap(s_pad), wrap(d_pad)
    idx_all = np.ascontiguousarray(
        np.stack([sw, dw], axis=1).transpose(2, 0, 1, 3).reshape(P, -1)
    )
    return ef_t, idx_all, blk_cnt, order, counts, seg_off


def unshard_core(out_em_core, order, counts, seg_off):
    """[n_blocks, P, c, D] fp16 edge-major -> [E, D] fp32 original order."""
    arr = np.asarray(out_em_core, dtype=np.float32)
    n_blocks = arr.shape[0]
    c = arr.shape[2]
    # slot b*block + ch*128 + p  ->  arr[b, p, ch, :]
    rows = arr.transpose(0, 2, 1, 3).reshape(n_blocks * c * P, D)
    out_c = np.empty((len(order), D), np.float32)
    pos = 0
    for k in range(4):
        sel = order[pos : pos + counts[k]]
        out_c[sel] = rows[seg_off[k] : seg_off[k] + counts[k]]
        pos += counts[k]
    return out_c


def _needed_blocks(counts, c=C):
    block = P * c
    return tuple(int(-(-int(n) // block)) if n else 1 for n in counts)


def kernel(node_feat, edge_feat, W, b, src, dst):
    out, _ = kernel_with_results(node_feat, edge_feat, W, b, src, dst)
    return out


def kernel_with_results(node_feat, edge_feat, W, b, src, dst, **spmd_kwargs):
    from concourse.bass_utils import run_bass_kernel_spmd

    node_feat = np.asarray(node_feat, dtype=np.float32)
    edge_feat = np.asarray(edge_feat, dtype=np.float32)
    W = np.asarray(W, dtype=np.float32)
    b = np.asarray(b, dtype=np.float32)
    src = np.ascontiguousarray(np.asarray(src, dtype=np.int32))
    dst = np.ascontiguousarray(np.asarray(dst, dtype=np.int32))

    p1, p2 = make_tables(node_feat, W, b)
    w3_t = np.ascontiguousarray(W[2 * D :, :].astype(F16))  # [128 f, 128 j]
    ef16 = edge_feat.astype(F16)

    class_blocks = list(CLASS_BLOCKS)
    per_core = []
    for i in range(N_CORES):
        lo = i * E_LOC
        sc, dc = src[lo : lo + E_LOC], dst[lo : lo + E_LOC]
        cls = (sc >= HALF).astype(np.int64) * 2 + (dc >= HALF).astype(np.int64)
        counts = np.bincount(cls, minlength=4)
        need = _needed_blocks(counts)
        class_blocks = [max(a, b_) for a, b_ in zip(class_blocks, need)]
        per_core.append((sc, dc))
    class_blocks = tuple(class_blocks)

    nc = _get_program(class_blocks)

    in_maps = []
    metas = []
    for i in range(N_CORES):
        lo = i * E_LOC
        sc, dc = per_core[i]
        ef_t, idx_all, blk_cnt, order, counts, seg_off = shard_core(
            ef16[lo : lo + E_LOC], sc, dc, class_blocks
        )
        metas.append((order, counts, seg_off))
        in_maps.append(
            {"p1": p1, "p2": p2, "efT": ef_t, "W3": w3_t, "idx_all": idx_all}
        )

    res = run_bass_kernel_spmd(nc, in_maps, list(range(N_CORES)), **spmd_kwargs)
    outs = []
    for i in range(N_CORES):
        order, counts, seg_off = metas[i]
        outs.append(unshard_core(res.results[i]["out_em"], order, counts, seg_off))
    return np.concatenate(outs, axis=0), res


# revision 22
# speedup vs baseline: 1.0764x; 1.0764x over previous
"""AggBondModule kernel v5 for Trainium2 (8 NeuronCores, SPMD edge-sharding).

out[e] = relu(concat(node_feat[src[e]], node_feat[dst[e]], edge_feat[e]) @ W + b)
       = relu(P1b[src[e]] + P2[dst[e]] + edge_feat[e] @ W3)

P1b = node_feat @ W1 + b, P2 = node_feat @ W2: host-projected fp16 node
table P12[n] = [P1b[n] | P2[n]] (512B/row).

Device per 2048-edge block, all edge-major (edge e = blk*2048 + ch*128 + p
lives at [partition p, cols ch*128:(ch+1)*128]):
  - 2x SWDGE dma_gather of 256B fp16 rows (elem_step selects the P1b/P2
    half), round-robin over 4 queues; queues drain concurrently (~140GB/s
    aggregate) once no other small-packet traffic competes.
  - PE: 16x (LoadStationary efT chunk [128f x 128e], moving W3 [128f x 128j])
    -> psum edge-major. No transpose anywhere (v3's DMA_TRANSPOSE emitted
    54k extra 256B packets that throttled the gather queues).
  - Scalar: 4x 512-wide ACT Copy psum -> ep_em fp16.
  - DVE: g1+g2, +ep, relu(max 0); store edge-major fp16.

Host-side 4-class machinery (int16 gather indices address the table in two
halves; edges class-sorted by (src < 32768, dst < 32768)).
"""

import sys

import numpy as np

sys.path.insert(0, "/opt/trn_rl_repo")

P = 128
D = 128
N_NODES = 50000
HALF = 32768
E_TOTAL = 400000
N_CORES = 8
C = 16  # 128-edge chunks per block
BLOCK = P * C  # 2048
CLASS_BLOCKS = (11, 6, 6, 3)
E_LOC = E_TOTAL // N_CORES

F16 = np.float16
N_QUEUES = 4
SCRATCH = 98304


def build_program(
    n_nodes=N_NODES,
    half=HALF,
    c=C,
    class_blocks=CLASS_BLOCKS,
    num_devices=N_CORES,
    n_queues=N_QUEUES,
):
    from concourse import bacc, mybir, tile

    block = P * c
    n_blocks = sum(class_blocks)
    e_pad = block * n_blocks
    idx_words = block // 16
    f32 = mybir.dt.float32
    fp16 = mybir.dt.float16
    i16 = mybir.dt.int16

    nc = bacc.Bacc(
        "TRN2",
        target_bir_lowering=False,
        debug=False,
        num_devices=num_devices,
        num_swdge_queues=n_queues,
        dynamic_dma_scratch_size=SCRATCH,
    )

    p1 = nc.dram_tensor("p1", [n_nodes, D], fp16, kind="ExternalInput").ap()
    p2 = nc.dram_tensor("p2", [n_nodes, D], fp16, kind="ExternalInput").ap()
    ef_t = nc.dram_tensor("efT", [P, e_pad], fp16, kind="ExternalInput").ap()
    w3_dram = nc.dram_tensor("W3", [P, D], fp16, kind="ExternalInput").ap()
    idx_dram = nc.dram_tensor(
        "idx_all", [P, n_blocks * 2 * idx_words], i16, kind="ExternalInput"
    ).ap()
    out_em = nc.dram_tensor("out_em", [n_blocks, P, c, D], fp16, kind="ExternalOutput").ap()

    # table views: [half rows] of the split P1b / P2 tables
    views = {
        (0, 0): p1[:half, :],
        (1, 0): p1[half:, :],
        (0, 1): p2[:half, :],
        (1, 1): p2[half:, :],
    }

    block_cls = []
    for cls_i, nb in enumerate(class_blocks):
        block_cls += [(cls_i >> 1 & 1, cls_i & 1)] * nb

    with tile.TileContext(nc) as tc:
        with (
            tc.tile_pool(name="const", bufs=1) as const_pool,
            tc.tile_pool(name="gath", bufs=6) as gath_pool,
            tc.tile_pool(name="feats", bufs=4) as feat_pool,
            tc.tile_pool(name="work", bufs=3) as work_pool,
            tc.tile_pool(name="opsum", bufs=2, space="PSUM") as ps_pool,
        ):
            # per-block idx loads, first blocks before anything else so the
            # gather pipeline starts immediately
            idx_all = const_pool.tile([P, n_blocks * 2 * idx_words], i16)
            w3_tile = const_pool.tile([P, D], fp16)
            for blk in range(n_blocks):
                lo = (2 * blk) * idx_words
                hi = (2 * blk + 2) * idx_words
                nc.sync.dma_start(idx_all[:, lo:hi], idx_dram[:, lo:hi])
                if blk == 3:
                    nc.sync.dma_start(w3_tile[:], w3_dram[:])

            for blk in range(n_blocks):
                src_hi, dst_hi = block_cls[blk]
                base = blk * block

                sidx = idx_all[:, (2 * blk) * idx_words : (2 * blk + 1) * idx_words]
                didx = idx_all[:, (2 * blk + 1) * idx_words : (2 * blk + 2) * idx_words]

                g1 = gath_pool.tile([P, c, D], fp16, tag="g1")
                g2 = gath_pool.tile([P, c, D], fp16, tag="g2")

                nc.gpsimd.dma_gather(
                    out_ap=g1[:],
                    in_ap=views[(src_hi, 0)],
                    idxs_ap=sidx,
                    num_idxs=block,
                    num_idxs_reg=block,
                    elem_size=D,
                    single_packet=False,
                    queue_num=(2 * blk) % n_queues,
                )
                nc.gpsimd.dma_gather(
                    out_ap=g2[:],
                    in_ap=views[(dst_hi, 1)],
                    idxs_ap=didx,
                    num_idxs=block,
                    num_idxs_reg=block,
                    elem_size=D,
                    single_packet=False,
                    queue_num=(2 * blk + 1) % n_queues,
                )

                sb_e = feat_pool.tile([P, block], fp16, tag="sb_e")
                nc.sync.dma_start(sb_e[:], ef_t[:, base : base + block])

                # EP = ef @ W3, edge-major: LS per 128-edge chunk, W3 moving
                psum = ps_pool.tile([P, block], f32, tag="ps")
                for ch in range(c):
                    nc.tensor.matmul(
                        psum[:, ch * D : (ch + 1) * D],
                        lhsT=sb_e[:, ch * P : (ch + 1) * P],
                        rhs=w3_tile[:],
                        start=True,
                        stop=True,
                    )
                ep_em = feat_pool.tile([P, block], fp16, tag="ep_em")
                for g in range(block // 512):
                    nc.scalar.activation(
                        ep_em[:, g * 512 : (g + 1) * 512],
                        psum[:, g * 512 : (g + 1) * 512],
                        mybir.ActivationFunctionType.Copy,
                    )

                t1 = work_pool.tile([P, c, D], fp16, tag="t1")
                nc.vector.tensor_tensor(t1[:], g1[:], g2[:], mybir.AluOpType.add)
                out_sb = work_pool.tile([P, c, D], fp16, tag="out_sb")
                nc.vector.tensor_tensor(out_sb[:], t1[:], ep_em[:], mybir.AluOpType.add)
                nc.vector.tensor_scalar_max(out_sb[:], out_sb[:], 0.0)

                nc.sync.dma_start(out_em[blk], out_sb[:])

    nc.compile()
    return nc


_PROGRAM_CACHE = {}


def _get_program(class_blocks=CLASS_BLOCKS):
    key = tuple(class_blocks)
    if key not in _PROGRAM_CACHE:
        _PROGRAM_CACHE[key] = build_program(class_blocks=key)
    return _PROGRAM_CACHE[key]


def make_tables(node_feat, W, b):
    """P1b[n] = node_feat[n] @ W1 + b, P2[n] = node_feat[n] @ W2, fp16."""
    p1 = node_feat @ W[:D, :] + b
    p2 = node_feat @ W[D : 2 * D, :]
    return (
        np.ascontiguousarray(p1.astype(F16)),
        np.ascontiguousarray(p2.astype(F16)),
    )


def shard_core(edge_feat_c, src_c, dst_c, class_blocks, half=HALF, c=C):
    """Classify/permute/pad one core's edges. Returns fp16 efT + wrapped idx."""
    block = P * c
    idx_words = block // 16
    n_blocks = sum(class_blocks)
    e_pad = block * n_blocks
    cls = (src_c >= half).astype(np.int64) * 2 + (dst_c >= half).astype(np.int64)
    # sort by (class, src): the src-gather then reads near-ascending table
    # rows (HBM row-buffer locality); dst stays random.
    order = np.lexsort((src_c, cls))
    counts = np.bincount(cls, minlength=4)
    seg_off = np.concatenate([[0], np.cumsum(np.asarray(class_blocks) * block)])

    ef_pad = np.zeros((e_pad, D), F16)
    s_pad = np.zeros((e_pad,), np.int16)
    d_pad = np.zeros((e_pad,), np.int16)
    src_reb = np.where(src_c >= half, src_c - half, src_c).astype(np.int16)
    dst_reb = np.where(dst_c >= half, dst_c - half, dst_c).astype(np.int16)

    pos = 0
    for k in range(4):
        sel = order[pos : pos + counts[k]]
        off = seg_off[k]
        ef_pad[off : off + counts[k]] = edge_feat_c[sel]
        s_pad[off : off + counts[k]] = src_reb[sel]
        d_pad[off : off + counts[k]] = dst_reb[sel]
        pos += counts[k]

    ef_t = np.ascontiguousarray(ef_pad.T)  # [128, e_pad] fp16

    def wrap(v):
        w16 = v.reshape(n_blocks, idx_words, 16).transpose(0, 2, 1)
        return np.ascontiguousarray(np.tile(w16, (1, P // 16, 1)))

    blk_cnt = np.full((n_blocks,), block, np.int32)

    sw, dw = wrap(s_pad), wrap(d_pad)
    idx_all = np.ascontiguousarray(
        np.stack([sw, dw], axis=1).transpose(2, 0, 1, 3).reshape(P, -1)
    )
    return ef_t, idx_all, blk_cnt, order, counts, seg_off


def unshard_core(out_em_core, order, counts, seg_off):
    """[n_blocks, P, c, D] fp16 edge-major -> [E, D] fp32 original order."""
    arr = np.asarray(out_em_core, dtype=np.float32)
    n_blocks = arr.shape[0]
    c = arr.shape[2]
    # slot b*block + ch*128 + p  ->  arr[b, p, ch, :]
    rows = arr.transpose(0, 2, 1, 3).reshape(n_blocks * c * P, D)
    out_c = np.empty((len(order), D), np.float32)
    pos = 0
    for k in range(4):
        sel = order[pos : pos + counts[k]]
        out_c[sel] = rows[seg_off[k] : seg_off[k] + counts[k]]
        pos += counts[k]
    return out_c


def _needed_blocks(counts, c=C):
    block = P * c
    return tuple(int(-(-int(n) // block)) if n else 1 for n in counts)


def kernel(node_feat, edge_feat, W, b, src, dst):
    out, _ = kernel_with_results(node_feat, edge_feat, W, b, src, dst)
    return out


def kernel_with_results(node_feat, edge_feat, W, b, src, dst, **spmd_kwargs):
    from concourse.bass_utils import run_bass_kernel_spmd

    node_feat = np.asarray(node_feat, dtype=np.float32)
    edge_feat = np.asarray(edge_feat, dtype=np.float32)
    W = np.asarray(W, dtype=np.float32)
    b = np.asarray(b, dtype=np.float32)
    src = np.ascontiguousarray(np.asarray(src, dtype=np.int32))
    dst = np.ascontiguousarray(np.asarray(dst, dtype=np.int32))

    p1, p2 = make_tables(node_feat, W, b)
    w3_t = np.ascontiguousarray(W[2 * D :, :].astype(F16))  # [128 f, 128 j]
    ef16 = edge_feat.astype(F16)

    class_blocks = list(CLASS_BLOCKS)
    per_core = []
    for i in range(N_CORES):
        lo = i * E_LOC
        sc, dc = src[lo : lo + E_LOC], dst[lo : lo + E_LOC]
        cls = (sc >= HALF).astype(np.int64) * 2 + (dc >= HALF).astype(np.int64)
        counts = np.bincount(cls, minlength=4)
        need = _needed_blocks(counts)
        class_blocks = [max(a, b_) for a, b_ in zip(class_blocks, need)]
        per_core.append((sc, dc))
    class_blocks = tuple(class_blocks)

    nc = _get_program(class_blocks)

    in_maps = []
    metas = []
    for i in range(N_CORES):
        lo = i * E_LOC
        sc, dc = per_core[i]
        ef_t, idx_all, blk_cnt, order, counts, seg_off = shard_core(
            ef16[lo : lo + E_LOC], sc, dc, class_blocks
        )
        metas.append((order, counts, seg_off))
        in_maps.append(
            {"p1": p1, "p2": p2, "efT": ef_t, "W3": w3_t, "idx_all": idx_all}
        )

    res = run_bass_kernel_spmd(nc, in_maps, list(range(N_CORES)), **spmd_kwargs)
    outs = []
    for i in range(N_CORES):
        order, counts, seg_off = metas[i]
        outs.append(unshard_core(res.results[i]["out_em"], order, counts, seg_off))
    return np.concatenate(outs, axis=0), res
